# revision 17
# baseline (speedup 1.0000x reference)
"""Trainium2 Bass kernel for nn_DeepLatent loss (chamfer + L2 of a per-point MLP).

Strategy (8 cores, data-parallel over batch B=32 -> 4 samples/core):
  Per core, per sample s (channel-major layout: activations stored [C, Npoints]):
    h1 = relu(W1o.T @ obs^T + latbias)        latbias = W1lat.T @ latent + b1
    h2 = relu(W2.T @ h1 + b2)
    h3 = relu(W3.T @ h2 + b3)
    delta = W4.T @ h3                         est = obs + delta + b4
  Chamfer via augmented grams (6 contraction rows: 3 coords + 3 aux):
    G [n,m]  = gt_n . est_m - |est_m|^2/2     (aux lhsT rows = -0.5, rhs rows = est^2)
    G'[m,n]  = est_m . gt_n - |gt_n|^2/2
    min_m d2[n,m] = |gt_n|^2 - 2 max_m G[n,m]
  The DVE max-reduce of the 64 [128,1024] gram PSUM tiles (~1.19us each) is the
  hard bottleneck (PSUM is readable only by DVE/ACT at 1 elem/lane/cycle and ACT
  cannot max) -- everything else is scheduled to hide beneath it:
    - PE: L1 2-way row-tiled (K=3), dense MM stream, gram of sample s-1
      interleaved into MLP of sample s to stay HAM-warm.
    - ACT: relus/est/est^2 only.
    - gt^2, gt-pack and sum|gt|^2 are host-side input marshaling; the cross
      term sum(gt*est) runs on a [96,128] repack (one tiny DVE op) instead of
      a 4.4us [3,4096] pass.
  Per-core partials are combined on the host.
"""

import ml_dtypes
import numpy as np
from contextlib import ExitStack

import concourse.bass as bass
import concourse.bacc as bacc
import concourse.mybir as mybir
import concourse.tile as tile
from concourse.bass_utils import run_bass_kernel_spmd

F32 = mybir.dt.float32
BF16 = mybir.dt.bfloat16
FP8 = mybir.dt.float8e4
AX = mybir.AxisListType
OP = mybir.AluOpType
ACTF = mybir.ActivationFunctionType
DR = mybir.MatmulPerfMode.DoubleRow
W8SCALE = 16.0  # fp8 weights stored x16 to stay clear of e4m3 subnormals

B, N, L = 32, 1024, 256
NCORES = 8
BS = B // NCORES  # samples per core
NT = N // 128     # n-tiles per sample

# test.py hooks
TRACE = False
LAST = None


def build_program():
    nc = bacc.Bacc()

    # host-pretransposed layouts: every DMA below is inner-contiguous
    obs_d = nc.dram_tensor("obs_t", [3, BS, N], BF16, kind="ExternalInput")[:]
    gt_d = nc.dram_tensor("gt_t", [3, BS, N], BF16, kind="ExternalInput")[:]
    gt2_d = nc.dram_tensor("gt2_t", [3, BS, N], BF16, kind="ExternalInput")[:]
    gtpk_d = nc.dram_tensor("gtpk", [96, 128], BF16, kind="ExternalInput")[:]
    latb_d = nc.dram_tensor("latb", [128, 4, BS], F32, kind="ExternalInput")[:]
    W1od = nc.dram_tensor("w1o", [35, 512], BF16, kind="ExternalInput")[:]
    W2d = nc.dram_tensor("w2p", [128, 4, 512], FP8, kind="ExternalInput")[:]
    b2d = nc.dram_tensor("b2p", [128, 4], F32, kind="ExternalInput")[:]
    W3d = nc.dram_tensor("w3p", [128, 4, 256], FP8, kind="ExternalInput")[:]
    b3d = nc.dram_tensor("b3p", [128, 2], F32, kind="ExternalInput")[:]
    W4d = nc.dram_tensor("w4p", [128, 2, 16], FP8, kind="ExternalInput")[:]
    b4d = nc.dram_tensor("b4p", [3, 1], F32, kind="ExternalInput")[:]
    out_d = nc.dram_tensor("partials", [1, 8], F32, kind="ExternalOutput")[:]

    with tile.TileContext(nc) as tc, ExitStack() as ctx:
        singles = ctx.enter_context(tc.tile_pool(name="singles", bufs=1))

        def fixed(shape, name, dtype=F32):
            return singles.tile(shape, dtype, tag=name, name=name)

        # ---------- fixed tiles ----------
        w1oR = fixed([35, 512], "w1oR", BF16)
        w2t = fixed([128, 4, 512], "w2t", FP8)
        w3t = fixed([128, 4, 256], "w3t", FP8)
        w4t = fixed([128, 2, 16], "w4t", FP8)
        b2t = fixed([128, 4], "b2t")
        b3t = fixed([128, 2], "b3t")
        b4p = fixed([3, 1], "b4p")
        ones_c = fixed([128, 1], "ones_c")
        latb = fixed([128, 4, BS], "latb")
        Pe = fixed([3, BS, N], "Pe", BF16)
        Pe2s = fixed([3, N], "Pe2s", BF16)
        SES = fixed([3, BS], "SES")
        gtpk = fixed([96, 128], "gtpk", BF16)
        epk = fixed([96, 128], "epk", BF16)
        eprod = fixed([96, 128], "eprod", BF16)
        M1 = fixed([128, BS * NT], "M1")
        M2 = fixed([128, BS * NT], "M2")
        Ft = fixed([128, 8], "Ft")
        outs = fixed([1, 8], "outs")
        # gram operand registers, whole-batch: [128, BS*N]
        #   A: rows {0-2,32-34}=gt, {3-5,35-37}=-0.5   (dir1 lhsT)
        #   D: rows {0-2,32-34}=gt, {3-5,35-37}=gt^2   (dir2 rhs)
        #   B: rows {0-2,32-34}=est, {3-5,35-37}=-0.5  (dir2 lhsT)
        #   C: rows {0-2,32-34}=est, {3-5,35-37}=est^2 (dir1 rhs)
        A_ = fixed([128, BS * N], "Areg", BF16)
        B_ = fixed([128, BS * N], "Breg", BF16)
        C_ = fixed([128, BS * N], "Creg", BF16)
        D_ = fixed([128, BS * N], "Dreg", BF16)

        otp = ctx.enter_context(tc.tile_pool(name="obsR", bufs=2))
        h1p = ctx.enter_context(tc.tile_pool(name="h1", bufs=2))
        h2p = ctx.enter_context(tc.tile_pool(name="h2", bufs=2))
        h3p = ctx.enter_context(tc.tile_pool(name="h3", bufs=2))
        psA = ctx.enter_context(tc.tile_pool(name="psA", bufs=2, space="PSUM"))
        psG = ctx.enter_context(tc.tile_pool(name="psG", bufs=2, space="PSUM"))

        # ---------- startup ----------
        # PE warmup: ~5us of dummy matmuls so HAM reaches K=8/8 before L1;
        # runs while the weight DMAs stream in.
        wtile = fixed([3, 512], "wtile", BF16)
        nc.vector.memset(wtile, 0.125)
        for i in range(14):
            wps = psG.tile([128, 1024], F32, tag="g", name=f"warm{i}")
            nc.tensor.matmul(wps[:, 0:512], wtile[:, 0:128], wtile[:, :],
                             start=True, stop=True)
        # L1-critical path on the sync queue
        nc.sync.dma_start(out=w1oR, in_=W1od)
        nc.sync.dma_start(out=latb, in_=latb_d)
        # later-layer weights on the scalar queue (ACT is idle at startup)
        nc.scalar.dma_start(out=w2t, in_=W2d)
        nc.scalar.dma_start(out=b2t, in_=b2d)
        nc.scalar.dma_start(out=w3t, in_=W3d)
        nc.scalar.dma_start(out=b3t, in_=b3d)
        nc.scalar.dma_start(out=w4t, in_=W4d)
        nc.scalar.dma_start(out=b4p, in_=b4d)
        nc.scalar.dma_start(out=gtpk, in_=gtpk_d)
        # gt-side gram registers straight from DRAM (whole batch); obsR(0) is
        # issued first on this queue by the main loop below.
        def load_gt_regs():
            for g in range(2):
                nc.gpsimd.dma_start(out=A_[32 * g:32 * g + 3, :],
                                    in_=gt_d.rearrange("c s n -> c (s n)"))
                nc.gpsimd.dma_start(out=D_[32 * g:32 * g + 3, :],
                                    in_=gt_d.rearrange("c s n -> c (s n)"))
                nc.gpsimd.dma_start(out=D_[32 * g + 3:32 * g + 6, :],
                                    in_=gt2_d.rearrange("c s n -> c (s n)"))
            for g in range(2):
                nc.gpsimd.dma_start(out=A_[32 * g + 3:32 * g + 6, :], in_=neghalf)
                nc.gpsimd.dma_start(out=B_[32 * g + 3:32 * g + 6, :], in_=neghalf)
        nc.vector.memset(ones_c, 1.0)
        nc.vector.memset(Ft, 0.0)
        # aux rows sit at partition base 3/35: engine APs must be 32-aligned,
        # so memset a staging band and DMA it into place.
        neghalf = fixed([3, BS * N], "neghalf", BF16)
        nc.vector.memset(neghalf, -0.5)

        # ---------- per-sample gram (generator; interleaved with next MLP) ----------
        def gram_rounds(s):
            o = s * N
            for lhs_reg, rhs_reg, Mt in ((B_, D_, M2), (A_, C_, M1)):
                for t in range(NT):
                    g = t % 2
                    gp = psG.tile([128, 1024], F32, tag="g", name=f"gp{s}_{t}")
                    for j in range(2):
                        nc.tensor.matmul(
                            gp[:, 512 * j:512 * (j + 1)],
                            lhs_reg[32 * g:32 * g + 6, o + 128 * t:o + 128 * (t + 1)],
                            rhs_reg[32 * g:32 * g + 6, o + 512 * j:o + 512 * (j + 1)],
                            start=True, stop=True)
                    nc.vector.tensor_reduce(
                        out=Mt[:, NT * s + t:NT * s + t + 1], in_=gp[:, :],
                        axis=AX.X, op=OP.max)
                    yield

        def advance(it, n=1):
            if it is not None:
                for _ in range(n):
                    next(it, None)

        def load_obsR(s):
            obsR = otp.tile([35, N], BF16, tag="obsR", name=f"obsR{s}")
            for g in range(2):
                nc.gpsimd.dma_start(out=obsR[32 * g:32 * g + 3, :], in_=obs_d[:, s, :])
            return obsR

        # ---------- per-sample MLP ----------
        def mlp(s, obsR, hooks):
            # L1: 2-way row-tiled (c0|c1 then c2|c3 concurrent on row groups 0/32)
            h1t = h1p.tile([128, 4, N], FP8, tag="h1", name=f"h1_{s}")
            for cp in range(2):
                pss = []
                for g in range(2):
                    c = 2 * cp + g
                    ps = psA.tile([128, N], F32, tag="a", name=f"l1ps{s}_{c}")
                    for j in range(2):
                        nc.tensor.matmul(ps[:, 512 * j:512 * (j + 1)],
                                         w1oR[32 * g:32 * g + 3, 128 * c:128 * (c + 1)],
                                         obsR[32 * g:32 * g + 3, 512 * j:512 * (j + 1)],
                                         start=True, stop=True,
                                         tile_position=(32 * g, 0))
                    pss.append((c, ps))
                for c, ps in pss:
                    nc.scalar.activation(h1t[:, c, :], ps[:, :], ACTF.Relu,
                                         bias=latb[:, c, s:s + 1])
                    advance(hooks)

            h2t = h2p.tile([128, 4, N], FP8, tag="h2", name=f"h2_{s}")
            for c in range(4):
                ps = psA.tile([128, N], F32, tag="a", name=f"l2ps{s}_{c}")
                for j in range(2):
                    for kp in range(2):
                        nc.tensor.matmul(ps[:, 512 * j:512 * (j + 1)],
                                         w2t[:, 2 * kp:2 * kp + 2, 128 * c:128 * (c + 1)],
                                         h1t[:, 2 * kp:2 * kp + 2, 512 * j:512 * (j + 1)],
                                         start=(kp == 0), stop=(kp == 1),
                                         perf_mode=DR)
                nc.scalar.activation(h2t[:, c, :], ps[:, :], ACTF.Relu,
                                     bias=b2t[:, c:c + 1], scale=1.0 / W8SCALE)
                advance(hooks)

            h3t = h3p.tile([128, 2, N], FP8, tag="h3", name=f"h3_{s}")
            for c in range(2):
                ps = psA.tile([128, N], F32, tag="a", name=f"l3ps{s}_{c}")
                for j in range(2):
                    for kp in range(2):
                        nc.tensor.matmul(ps[:, 512 * j:512 * (j + 1)],
                                         w3t[:, 2 * kp:2 * kp + 2, 128 * c:128 * (c + 1)],
                                         h2t[:, 2 * kp:2 * kp + 2, 512 * j:512 * (j + 1)],
                                         start=(kp == 0), stop=(kp == 1),
                                         perf_mode=DR)
                nc.scalar.activation(h3t[:, c, :], ps[:, :], ACTF.Relu,
                                     bias=b3t[:, c:c + 1], scale=1.0 / W8SCALE)
                advance(hooks, 2)

            ps4 = psA.tile([128, N], F32, tag="a", name=f"l4ps{s}")
            for j in range(2):
                nc.tensor.matmul(ps4[0:3, 512 * j:512 * (j + 1)],
                                 w4t[:, :, 0:3],
                                 h3t[:, :, 512 * j:512 * (j + 1)],
                                 start=True, stop=True, perf_mode=DR)
            # Pe := delta + b4, then obs is added by a DMA-accum (frees the PE
            # of the identity matmuls)
            nc.scalar.activation(Pe[:, s, :], ps4[0:3, :], ACTF.Identity,
                                 bias=b4p[:, 0:1], scale=1.0 / W8SCALE)
            advance(hooks, 2)
            o = s * N
            nc.gpsimd.dma_start(out=Pe[:, s, :], in_=obs_d[:, s, :],
                                accum_op=OP.add)
            for g in range(2):
                nc.sync.dma_start(out=B_[32 * g:32 * g + 3, o:o + N], in_=Pe[:, s, :])
            nc.scalar.activation(Pe2s[:, :], Pe[:, s, :], ACTF.Square,
                                 accum_out=SES[:, s:s + 1])
            for g in range(2):
                nc.gpsimd.dma_start(out=C_[32 * g:32 * g + 3, o:o + N], in_=Pe[:, s, :])
                nc.gpsimd.dma_start(out=C_[32 * g + 3:32 * g + 6, o:o + N], in_=Pe2s[:, :])
            for c in range(3):
                nc.gpsimd.dma_start(
                    out=epk[24 * s + 8 * c:24 * s + 8 * (c + 1), :],
                    in_=Pe[c:c + 1, s, :])
            advance(hooks, 2)

        pending = None
        obsR = load_obsR(0)
        load_gt_regs()
        for s in range(BS):
            obsR_next = load_obsR(s + 1) if s + 1 < BS else None
            mlp(s, obsR, pending)
            if pending is not None:
                for _ in pending:
                    pass
            pending = gram_rounds(s)
            obsR = obsR_next
        if pending is not None:
            for _ in pending:
                pass

        # ---------- finale ----------
        nc.vector.tensor_reduce(out=Ft[:, 0:1], in_=M1[:, :], axis=AX.X, op=OP.add)
        nc.vector.tensor_reduce(out=Ft[:, 1:2], in_=M2[:, :], axis=AX.X, op=OP.add)
        nc.vector.tensor_reduce(out=Ft[0:3, 3:4], in_=SES[:, :], axis=AX.X, op=OP.add)
        nc.vector.scalar_tensor_tensor(out=eprod[:, :], in0=gtpk[:, :],
                                       scalar=0.0, in1=epk[:, :],
                                       op0=OP.add, op1=OP.mult,
                                       accum_out=Ft[0:96, 4:5])

        fps = psG.tile([128, 1024], F32, tag="g", name="fps")
        nc.tensor.matmul(fps[0:1, 0:8], ones_c[:, :], Ft[:, :],
                         start=True, stop=True)
        nc.scalar.activation(outs[:, :], fps[0:1, 0:8], ACTF.Copy)
        nc.sync.dma_start(out=out_d, in_=outs)

    nc.compile()
    return nc


_program_cache = []


def kernel(**inputs):
    global LAST
    if not _program_cache:
        _program_cache.append(build_program())
    nc = _program_cache[0]

    def f32(x):
        return np.ascontiguousarray(np.asarray(x, dtype=np.float32))

    W1 = np.asarray(inputs["W1"], np.float32)
    W2 = np.asarray(inputs["W2"], np.float32)
    W3 = np.asarray(inputs["W3"], np.float32)
    W4 = np.asarray(inputs["W4"], np.float32)
    w1o = np.zeros((35, 512), np.float32)
    w1o[0:3] = W1[0:3]
    w1o[32:35] = W1[0:3]
    shared = {
        "w1o": np.ascontiguousarray(w1o.astype(ml_dtypes.bfloat16)),
        "w2p": np.ascontiguousarray((W2 * W8SCALE).reshape(4, 128, 512).transpose(1, 0, 2).astype(ml_dtypes.float8_e4m3)),
        "b2p": f32(np.asarray(inputs["b2"], np.float32).reshape(4, 128).T),
        "w3p": np.ascontiguousarray((W3 * W8SCALE).reshape(4, 128, 256).transpose(1, 0, 2).astype(ml_dtypes.float8_e4m3)),
        "b3p": f32(np.asarray(inputs["b3"], np.float32).reshape(2, 128).T),
        "w4p": np.ascontiguousarray(np.pad((W4 * W8SCALE).reshape(2, 128, 3).transpose(1, 0, 2),
                                           ((0, 0), (0, 0), (0, 13))).astype(ml_dtypes.float8_e4m3)),
        "b4p": f32(np.asarray(inputs["b4"], np.float32).reshape(3, 1)),
    }
    obs = np.asarray(inputs["obs"], np.float32)
    gt = np.asarray(inputs["obs_gt"], np.float32)
    lat = np.asarray(inputs["latent"], np.float32)
    in_maps = []
    s_gt2 = float((gt.astype(np.float64) ** 2).sum())
    for c in range(NCORES):
        sl = slice(c * BS, (c + 1) * BS)
        m = dict(shared)
        gt_t = np.ascontiguousarray(gt[sl].transpose(2, 0, 1).astype(ml_dtypes.bfloat16))
        m["obs_t"] = np.ascontiguousarray(obs[sl].transpose(2, 0, 1).astype(ml_dtypes.bfloat16))
        m["gt_t"] = gt_t
        m["gt2_t"] = np.ascontiguousarray(
            (gt[sl].transpose(2, 0, 1) ** 2).astype(ml_dtypes.bfloat16))
        # gtpk row = s*24 + c*8 + b, cols = 128-elem n-block b
        m["gtpk"] = np.ascontiguousarray(
            gt_t.transpose(1, 0, 2).reshape(BS * 3, 8, 128).reshape(96, 128))
        hb = lat[sl] @ np.asarray(inputs["W1"], np.float32)[3:259] \
            + np.asarray(inputs["b1"], np.float32)
        m["latb"] = f32(hb.T.reshape(4, 128, BS).transpose(1, 0, 2))
        in_maps.append(m)

    res = run_bass_kernel_spmd(nc, in_maps, core_ids=list(range(NCORES)),
                               trace=TRACE)
    LAST = res

    parts = np.stack([r["partials"][0] for r in res.results]).astype(np.float64)
    s_max1 = parts[:, 0].sum()
    s_max2 = parts[:, 1].sum()
    s_est2 = parts[:, 3].sum()
    s_cross = parts[:, 4].sum()
    chm = (s_gt2 - 2.0 * s_max1) / (B * N) + (s_est2 - 2.0 * s_max2) / (B * N)
    l2 = (s_gt2 - 2.0 * s_cross + s_est2) / (B * N * 3)
    loss = 0.2 * chm + 0.8 * l2
    return np.asarray(loss, dtype=np.float32)


# revision 18
# speedup vs baseline: 1.0104x; 1.0104x over previous
"""Trainium2 Bass kernel for nn_DeepLatent loss (chamfer + L2 of a per-point MLP).

Strategy (8 cores, data-parallel over batch B=32 -> 4 samples/core):
  Per core, per sample s (channel-major layout: activations stored [C, Npoints]):
    h1 = relu(W1o.T @ obs^T + latbias)        latbias = W1lat.T @ latent + b1
    h2 = relu(W2.T @ h1 + b2)
    h3 = relu(W3.T @ h2 + b3)
    delta = W4.T @ h3                         est = obs + delta + b4
  Chamfer via augmented grams (6 contraction rows: 3 coords + 3 aux):
    G [n,m]  = gt_n . est_m - |est_m|^2/2     (aux lhsT rows = -0.5, rhs rows = est^2)
    G'[m,n]  = est_m . gt_n - |gt_n|^2/2
    min_m d2[n,m] = |gt_n|^2 - 2 max_m G[n,m]
  The DVE max-reduce of the 64 [128,1024] gram PSUM tiles (~1.19us each) is the
  hard bottleneck (PSUM is readable only by DVE/ACT at 1 elem/lane/cycle and ACT
  cannot max) -- everything else is scheduled to hide beneath it:
    - PE: L1 2-way row-tiled (K=3), dense MM stream, gram of sample s-1
      interleaved into MLP of sample s to stay HAM-warm.
    - ACT: relus/est/est^2 only.
    - gt^2, gt-pack and sum|gt|^2 are host-side input marshaling; the cross
      term sum(gt*est) runs on a [96,128] repack (one tiny DVE op) instead of
      a 4.4us [3,4096] pass.
  Per-core partials are combined on the host.
"""

import ml_dtypes
import numpy as np
from contextlib import ExitStack

import concourse.bass as bass
import concourse.bacc as bacc
import concourse.mybir as mybir
import concourse.tile as tile
from concourse.bass_utils import run_bass_kernel_spmd

F32 = mybir.dt.float32
BF16 = mybir.dt.bfloat16
FP8 = mybir.dt.float8e4
AX = mybir.AxisListType
OP = mybir.AluOpType
ACTF = mybir.ActivationFunctionType
DR = mybir.MatmulPerfMode.DoubleRow
W8SCALE = 16.0  # fp8 weights stored x16 to stay clear of e4m3 subnormals

B, N, L = 32, 1024, 256
NCORES = 8
BS = B // NCORES  # samples per core
NT = N // 128     # n-tiles per sample

# test.py hooks
TRACE = False
LAST = None


def build_program():
    nc = bacc.Bacc()

    # host-pretransposed layouts: every DMA below is inner-contiguous
    obs_d = nc.dram_tensor("obs_t", [3, BS, N], BF16, kind="ExternalInput")[:]
    gt_d = nc.dram_tensor("gt_t", [3, BS, N], BF16, kind="ExternalInput")[:]
    gt2_d = nc.dram_tensor("gt2_t", [3, BS, N], BF16, kind="ExternalInput")[:]
    gtpk_d = nc.dram_tensor("gtpk", [96, 128], BF16, kind="ExternalInput")[:]
    latb_d = nc.dram_tensor("latb", [128, 4, BS], F32, kind="ExternalInput")[:]
    W1od = nc.dram_tensor("w1o", [35, 512], BF16, kind="ExternalInput")[:]
    W2d = nc.dram_tensor("w2p", [128, 4, 512], FP8, kind="ExternalInput")[:]
    b2d = nc.dram_tensor("b2p", [128, 4], F32, kind="ExternalInput")[:]
    W3d = nc.dram_tensor("w3p", [128, 4, 256], FP8, kind="ExternalInput")[:]
    b3d = nc.dram_tensor("b3p", [128, 2], F32, kind="ExternalInput")[:]
    W4d = nc.dram_tensor("w4p", [128, 2, 16], FP8, kind="ExternalInput")[:]
    b4d = nc.dram_tensor("b4p", [3, 1], F32, kind="ExternalInput")[:]
    out_d = nc.dram_tensor("partials", [1, 8], F32, kind="ExternalOutput")[:]

    with tile.TileContext(nc) as tc, ExitStack() as ctx:
        singles = ctx.enter_context(tc.tile_pool(name="singles", bufs=1))

        def fixed(shape, name, dtype=F32):
            return singles.tile(shape, dtype, tag=name, name=name)

        # ---------- fixed tiles ----------
        w1oR = fixed([35, 512], "w1oR", BF16)
        w2t = fixed([128, 4, 512], "w2t", FP8)
        w3t = fixed([128, 4, 256], "w3t", FP8)
        w4t = fixed([128, 2, 16], "w4t", FP8)
        b2t = fixed([128, 4], "b2t")
        b3t = fixed([128, 2], "b3t")
        b4p = fixed([3, 1], "b4p")
        ones_c = fixed([128, 1], "ones_c")
        latb = fixed([128, 4, BS], "latb")
        Pe = fixed([3, BS, N], "Pe", BF16)
        Pe2s = fixed([3, N], "Pe2s", BF16)
        SES = fixed([3, BS], "SES")
        gtpk = fixed([96, 128], "gtpk", BF16)
        epk = fixed([96, 128], "epk", BF16)
        eprod = fixed([96, 128], "eprod", BF16)
        M1 = fixed([128, BS * NT], "M1")
        M2 = fixed([128, BS * NT], "M2")
        Ft = fixed([128, 8], "Ft")
        outs = fixed([1, 8], "outs")
        # gram operand registers, whole-batch: [128, BS*N]
        #   A: rows {0-2,32-34}=gt, {3-5,35-37}=-0.5   (dir1 lhsT)
        #   D: rows {0-2,32-34}=gt, {3-5,35-37}=gt^2   (dir2 rhs)
        #   B: rows {0-2,32-34}=est, {3-5,35-37}=-0.5  (dir2 lhsT)
        #   C: rows {0-2,32-34}=est, {3-5,35-37}=est^2 (dir1 rhs)
        A_ = fixed([128, BS * N], "Areg", BF16)
        B_ = fixed([128, BS * N], "Breg", BF16)
        C_ = fixed([128, BS * N], "Creg", BF16)
        D_ = fixed([128, BS * N], "Dreg", BF16)

        otp = ctx.enter_context(tc.tile_pool(name="obsR", bufs=2))
        h1p = ctx.enter_context(tc.tile_pool(name="h1", bufs=2))
        h2p = ctx.enter_context(tc.tile_pool(name="h2", bufs=2))
        h3p = ctx.enter_context(tc.tile_pool(name="h3", bufs=2))
        psA = ctx.enter_context(tc.tile_pool(name="psA", bufs=2, space="PSUM"))
        psG = ctx.enter_context(tc.tile_pool(name="psG", bufs=2, space="PSUM"))

        # ---------- startup ----------
        # PE warmup: ~5us of dummy matmuls so HAM reaches K=8/8 before L1;
        # runs while the weight DMAs stream in.
        wtile = fixed([3, 512], "wtile", BF16)
        nc.vector.memset(wtile, 0.125)
        for i in range(14):
            wps = psG.tile([128, 1024], F32, tag="g", name=f"warm{i}")
            nc.tensor.matmul(wps[:, 0:512], wtile[:, 0:128], wtile[:, :],
                             start=True, stop=True)
        # L1-critical path on the sync queue
        nc.sync.dma_start(out=w1oR, in_=W1od)
        nc.sync.dma_start(out=latb, in_=latb_d)
        # later-layer weights on the scalar queue (ACT is idle at startup)
        nc.scalar.dma_start(out=w2t, in_=W2d)
        nc.scalar.dma_start(out=b2t, in_=b2d)
        nc.scalar.dma_start(out=w3t, in_=W3d)
        nc.scalar.dma_start(out=b3t, in_=b3d)
        nc.scalar.dma_start(out=w4t, in_=W4d)
        nc.scalar.dma_start(out=b4p, in_=b4d)
        nc.scalar.dma_start(out=gtpk, in_=gtpk_d)
        # gt-side gram registers straight from DRAM (whole batch); obsR(0) is
        # issued first on this queue by the main loop below.
        def load_gt_regs():
            for g in range(2):
                nc.gpsimd.dma_start(out=A_[32 * g:32 * g + 3, :],
                                    in_=gt_d.rearrange("c s n -> c (s n)"))
                nc.gpsimd.dma_start(out=D_[32 * g:32 * g + 3, :],
                                    in_=gt_d.rearrange("c s n -> c (s n)"))
                nc.gpsimd.dma_start(out=D_[32 * g + 3:32 * g + 6, :],
                                    in_=gt2_d.rearrange("c s n -> c (s n)"))
            for g in range(2):
                nc.gpsimd.dma_start(out=A_[32 * g + 3:32 * g + 6, :], in_=neghalf)
                nc.gpsimd.dma_start(out=B_[32 * g + 3:32 * g + 6, :], in_=neghalf)
        nc.vector.memset(ones_c, 1.0)
        nc.vector.memset(Ft, 0.0)
        # aux rows sit at partition base 3/35: engine APs must be 32-aligned,
        # so memset a staging band and DMA it into place.
        neghalf = fixed([3, BS * N], "neghalf", BF16)
        nc.vector.memset(neghalf, -0.5)

        # ---------- per-sample gram (generator; interleaved with next MLP) ----------
        def gram_rounds(s):
            o = s * N
            for lhs_reg, rhs_reg, Mt in ((B_, D_, M2), (A_, C_, M1)):
                for t in range(NT):
                    g = t % 2
                    gp = psG.tile([128, 1024], F32, tag="g", name=f"gp{s}_{t}")
                    for j in range(2):
                        nc.tensor.matmul(
                            gp[:, 512 * j:512 * (j + 1)],
                            lhs_reg[32 * g:32 * g + 6, o + 128 * t:o + 128 * (t + 1)],
                            rhs_reg[32 * g:32 * g + 6, o + 512 * j:o + 512 * (j + 1)],
                            start=True, stop=True)
                    nc.vector.tensor_reduce(
                        out=Mt[:, NT * s + t:NT * s + t + 1], in_=gp[:, :],
                        axis=AX.X, op=OP.max)
                    yield

        def advance(it, n=1):
            if it is not None:
                for _ in range(n):
                    next(it, None)

        def load_obsR(s):
            obsR = otp.tile([35, N], BF16, tag="obsR", name=f"obsR{s}")
            for g in range(2):
                nc.gpsimd.dma_start(out=obsR[32 * g:32 * g + 3, :], in_=obs_d[:, s, :])
            return obsR

        # ---------- per-sample MLP ----------
        def mlp(s, obsR, hooks):
            # L1: 2-way row-tiled (c0|c1 then c2|c3 concurrent on row groups 0/32)
            h1t = h1p.tile([128, 4, N], FP8, tag="h1", name=f"h1_{s}")
            for cp in range(2):
                pss = []
                for g in range(2):
                    c = 2 * cp + g
                    ps = psA.tile([128, N], F32, tag="a", name=f"l1ps{s}_{c}")
                    for j in range(2):
                        nc.tensor.matmul(ps[:, 512 * j:512 * (j + 1)],
                                         w1oR[32 * g:32 * g + 3, 128 * c:128 * (c + 1)],
                                         obsR[32 * g:32 * g + 3, 512 * j:512 * (j + 1)],
                                         start=True, stop=True,
                                         tile_position=(32 * g, 0))
                    pss.append((c, ps))
                for c, ps in pss:
                    nc.scalar.activation(h1t[:, c, :], ps[:, :], ACTF.Relu,
                                         bias=latb[:, c, s:s + 1])
                    advance(hooks)

            h2t = h2p.tile([128, 4, N], FP8, tag="h2", name=f"h2_{s}")
            for c in range(4):
                ps = psA.tile([128, N], F32, tag="a", name=f"l2ps{s}_{c}")
                for j in range(2):
                    for kp in range(2):
                        nc.tensor.matmul(ps[:, 512 * j:512 * (j + 1)],
                                         w2t[:, 2 * kp:2 * kp + 2, 128 * c:128 * (c + 1)],
                                         h1t[:, 2 * kp:2 * kp + 2, 512 * j:512 * (j + 1)],
                                         start=(kp == 0), stop=(kp == 1),
                                         perf_mode=DR)
                nc.scalar.activation(h2t[:, c, :], ps[:, :], ACTF.Relu,
                                     bias=b2t[:, c:c + 1], scale=1.0 / W8SCALE)
                advance(hooks)

            h3t = h3p.tile([128, 2, N], FP8, tag="h3", name=f"h3_{s}")
            for c in range(2):
                ps = psA.tile([128, N], F32, tag="a", name=f"l3ps{s}_{c}")
                for j in range(2):
                    for kp in range(2):
                        nc.tensor.matmul(ps[:, 512 * j:512 * (j + 1)],
                                         w3t[:, 2 * kp:2 * kp + 2, 128 * c:128 * (c + 1)],
                                         h2t[:, 2 * kp:2 * kp + 2, 512 * j:512 * (j + 1)],
                                         start=(kp == 0), stop=(kp == 1),
                                         perf_mode=DR)
                nc.scalar.activation(h3t[:, c, :], ps[:, :], ACTF.Relu,
                                     bias=b3t[:, c:c + 1], scale=1.0 / W8SCALE)
                advance(hooks, 2)

            ps4 = psA.tile([128, N], F32, tag="a", name=f"l4ps{s}")
            for j in range(2):
                nc.tensor.matmul(ps4[0:3, 512 * j:512 * (j + 1)],
                                 w4t[:, :, 0:3],
                                 h3t[:, :, 512 * j:512 * (j + 1)],
                                 start=True, stop=True, perf_mode=DR)
            # Pe := delta + b4, then obs is added by a DMA-accum (frees the PE
            # of the identity matmuls)
            nc.scalar.activation(Pe[:, s, :], ps4[0:3, :], ACTF.Identity,
                                 bias=b4p[:, 0:1], scale=1.0 / W8SCALE)
            advance(hooks, 2)
            o = s * N
            nc.gpsimd.dma_start(out=Pe[:, s, :], in_=obs_d[:, s, :],
                                accum_op=OP.add)
            for g in range(2):
                nc.gpsimd.dma_start(out=B_[32 * g:32 * g + 3, o:o + N], in_=Pe[:, s, :])
            nc.scalar.activation(Pe2s[:, :], Pe[:, s, :], ACTF.Square,
                                 accum_out=SES[:, s:s + 1])
            for g in range(2):
                nc.gpsimd.dma_start(out=C_[32 * g:32 * g + 3, o:o + N], in_=Pe[:, s, :])
                nc.gpsimd.dma_start(out=C_[32 * g + 3:32 * g + 6, o:o + N], in_=Pe2s[:, :])
            for c in range(3):
                nc.gpsimd.dma_start(
                    out=epk[24 * s + 8 * c:24 * s + 8 * (c + 1), :],
                    in_=Pe[c:c + 1, s, :])
            advance(hooks, 2)

        pending = None
        obsR = load_obsR(0)
        load_gt_regs()
        for s in range(BS):
            obsR_next = load_obsR(s + 1) if s + 1 < BS else None
            mlp(s, obsR, pending)
            if pending is not None:
                for _ in pending:
                    pass
            pending = gram_rounds(s)
            obsR = obsR_next
        if pending is not None:
            for _ in pending:
                pass

        # ---------- finale ----------
        nc.vector.tensor_reduce(out=Ft[:, 0:1], in_=M1[:, :], axis=AX.X, op=OP.add)
        nc.vector.tensor_reduce(out=Ft[:, 1:2], in_=M2[:, :], axis=AX.X, op=OP.add)
        nc.vector.tensor_reduce(out=Ft[0:3, 3:4], in_=SES[:, :], axis=AX.X, op=OP.add)
        nc.vector.scalar_tensor_tensor(out=eprod[:, :], in0=gtpk[:, :],
                                       scalar=0.0, in1=epk[:, :],
                                       op0=OP.add, op1=OP.mult,
                                       accum_out=Ft[0:96, 4:5])

        fps = psG.tile([128, 1024], F32, tag="g", name="fps")
        nc.tensor.matmul(fps[0:1, 0:8], ones_c[:, :], Ft[:, :],
                         start=True, stop=True)
        nc.scalar.activation(outs[:, :], fps[0:1, 0:8], ACTF.Copy)
        nc.sync.dma_start(out=out_d, in_=outs)

    nc.compile()
    return nc


_program_cache = []


def kernel(**inputs):
    global LAST
    if not _program_cache:
        _program_cache.append(build_program())
    nc = _program_cache[0]

    def f32(x):
        return np.ascontiguousarray(np.asarray(x, dtype=np.float32))

    W1 = np.asarray(inputs["W1"], np.float32)
    W2 = np.asarray(inputs["W2"], np.float32)
    W3 = np.asarray(inputs["W3"], np.float32)
    W4 = np.asarray(inputs["W4"], np.float32)
    w1o = np.zeros((35, 512), np.float32)
    w1o[0:3] = W1[0:3]
    w1o[32:35] = W1[0:3]
    shared = {
        "w1o": np.ascontiguousarray(w1o.astype(ml_dtypes.bfloat16)),
        "w2p": np.ascontiguousarray((W2 * W8SCALE).reshape(4, 128, 512).transpose(1, 0, 2).astype(ml_dtypes.float8_e4m3)),
        "b2p": f32(np.asarray(inputs["b2"], np.float32).reshape(4, 128).T),
        "w3p": np.ascontiguousarray((W3 * W8SCALE).reshape(4, 128, 256).transpose(1, 0, 2).astype(ml_dtypes.float8_e4m3)),
        "b3p": f32(np.asarray(inputs["b3"], np.float32).reshape(2, 128).T),
        "w4p": np.ascontiguousarray(np.pad((W4 * W8SCALE).reshape(2, 128, 3).transpose(1, 0, 2),
                                           ((0, 0), (0, 0), (0, 13))).astype(ml_dtypes.float8_e4m3)),
        "b4p": f32(np.asarray(inputs["b4"], np.float32).reshape(3, 1)),
    }
    obs = np.asarray(inputs["obs"], np.float32)
    gt = np.asarray(inputs["obs_gt"], np.float32)
    lat = np.asarray(inputs["latent"], np.float32)
    in_maps = []
    s_gt2 = float((gt.astype(np.float64) ** 2).sum())
    for c in range(NCORES):
        sl = slice(c * BS, (c + 1) * BS)
        m = dict(shared)
        gt_t = np.ascontiguousarray(gt[sl].transpose(2, 0, 1).astype(ml_dtypes.bfloat16))
        m["obs_t"] = np.ascontiguousarray(obs[sl].transpose(2, 0, 1).astype(ml_dtypes.bfloat16))
        m["gt_t"] = gt_t
        m["gt2_t"] = np.ascontiguousarray(
            (gt[sl].transpose(2, 0, 1) ** 2).astype(ml_dtypes.bfloat16))
        # gtpk row = s*24 + c*8 + b, cols = 128-elem n-block b
        m["gtpk"] = np.ascontiguousarray(
            gt_t.transpose(1, 0, 2).reshape(BS * 3, 8, 128).reshape(96, 128))
        hb = lat[sl] @ np.asarray(inputs["W1"], np.float32)[3:259] \
            + np.asarray(inputs["b1"], np.float32)
        m["latb"] = f32(hb.T.reshape(4, 128, BS).transpose(1, 0, 2))
        in_maps.append(m)

    res = run_bass_kernel_spmd(nc, in_maps, core_ids=list(range(NCORES)),
                               trace=TRACE)
    LAST = res

    parts = np.stack([r["partials"][0] for r in res.results]).astype(np.float64)
    s_max1 = parts[:, 0].sum()
    s_max2 = parts[:, 1].sum()
    s_est2 = parts[:, 3].sum()
    s_cross = parts[:, 4].sum()
    chm = (s_gt2 - 2.0 * s_max1) / (B * N) + (s_est2 - 2.0 * s_max2) / (B * N)
    l2 = (s_gt2 - 2.0 * s_cross + s_est2) / (B * N * 3)
    loss = 0.2 * chm + 0.8 * l2
    return np.asarray(loss, dtype=np.float32)
